# revision 13
# baseline (speedup 1.0000x reference)
"""2x2 average pool + per-channel affine on 8 TRN2 NeuronCores.

Problem: x (16, 64, 512, 512) f32 -> out (16, 64, 256, 256) f32
  out[b,c,i,j] = weight[c] * mean(x[b,c,2i:2i+2,2j:2j+2]) + bias[c]

Sharding: pure data parallel over batch. Core k gets batches [2k, 2k+1]
(128 images of 512x512 per core), weight/bias replicated.

Per-core layout: partition p = (b_local*64 + c) -> one full image per
partition. Each iteration DMAs 16 input rows per partition (32 KiB
contiguous, 4 MiB per dma_start), does the vertical pool with one
tensor_tensor add (row pairs are adjacent in the free dim), the
horizontal pool with a stride-2 tensor_tensor add, and the per-channel
affine on the scalar engine (scale/bias are per-partition [128,1]
scalars since partition == (b_local, c)).
"""

import numpy as np

import concourse.bacc as bacc
import concourse.bass as bass
import concourse.mybir as mybir
import concourse.tile as tile
from concourse.bass_utils import run_bass_kernel_spmd

N_CORES = 8
B, C, S = 16, 64, 512
B_LOC = B // N_CORES            # 2 batches per core
P = B_LOC * C                   # 128 partitions = one image per partition
IMG = S * S                     # 262144 input elems per image
OS = S // 2                     # 256
OUT_IMG = OS * OS               # 65536 output elems per image
ROWS_PER_ITER = 16              # input rows loaded per iteration
CHUNK = ROWS_PER_ITER * S       # 8192 elems per partition per load (32 KiB)
N_ITERS = IMG // CHUNK          # 32
OUT_CHUNK = CHUNK // 4          # 2048 elems per partition per store

FP32 = mybir.dt.float32

_nc_cache = None


def _build(reps=1, rows=8, ibufs=6, vbufs=3, hbufs=3, obufs=4,
           store_eng="scalar", split_load=1):
    # Bacc (not raw Bass): its finalize pass splits multi-sem waits into
    # event-semaphore instructions — TRN2 allows at most 1 wait per inst.
    # reps>1 repeats the full pass back-to-back in one NEFF (delta-timing).
    nc = bacc.Bacc("TRN2", target_bir_lowering=False, debug=False,
                   num_devices=N_CORES)
    chunk = rows * S             # input elems per partition per iteration
    n_iters = IMG // chunk
    out_chunk = chunk // 4

    x = nc.declare_dram_parameter("x", [P, IMG], FP32, isOutput=False)
    # affine[:, 0] = weight[c] / 4 (pool norm folded in), affine[:, 1] = bias[c]
    # (host-precomputed, already broadcast to the 128 partition images)
    affine = nc.declare_dram_parameter("affine", [P, 2], FP32, isOutput=False)
    out = nc.declare_dram_parameter("out", [P, OUT_IMG], FP32, isOutput=True)

    store = {"sync": nc.sync, "scalar": nc.scalar, "gpsimd": nc.gpsimd}[store_eng]

    with tile.TileContext(nc) as tc:
        with tc.tile_pool(name="consts", bufs=1) as cpool, \
             tc.tile_pool(name="ld", bufs=ibufs) as ipool, \
             tc.tile_pool(name="vmid", bufs=vbufs) as vpool, \
             tc.tile_pool(name="hmid", bufs=hbufs) as hpool, \
             tc.tile_pool(name="st", bufs=obufs) as opool:

            cb = cpool.tile([P, 2], FP32)
            nc.sync.dma_start(out=cb[:], in_=affine[:, :])
            s_ap = cb[:, 0:1]
            b_ap = cb[:, 1:2]

            for i in range(n_iters * reps):
                i = i % n_iters
                t = ipool.tile([P, chunk], FP32)
                if split_load == 1:
                    nc.sync.dma_start(out=t[:],
                                      in_=x[:, i * chunk:(i + 1) * chunk])
                else:
                    part = chunk // split_load
                    for s_ in range(split_load):
                        nc.sync.dma_start(
                            out=t[:, s_ * part:(s_ + 1) * part],
                            in_=x[:, i * chunk + s_ * part:
                                  i * chunk + (s_ + 1) * part])

                # vertical pool: rows 2r and 2r+1 sit at free-dim offsets
                # (2r*S, (2r+1)*S) -> contiguous-stride add
                tv = t[:].rearrange("p (r two w) -> p r two w", two=2, w=S)
                v = vpool.tile([P, chunk // 2], FP32)
                vv = v[:].rearrange("p (r w) -> p r w", w=S)
                nc.vector.tensor_add(vv, tv[:, :, 0, :], tv[:, :, 1, :])

                # horizontal pool: adjacent column pairs, stride-2 operands
                vh = v[:].rearrange("p (r j two) -> p r j two", two=2, j=OS)
                h = hpool.tile([P, out_chunk], FP32)
                hh = h[:].rearrange("p (r j) -> p r j", j=OS)
                nc.vector.tensor_add(hh, vh[:, :, :, 0], vh[:, :, :, 1])

                # per-channel affine on the scalar engine:
                # y = Identity(h * (w[c]/4) + bias[c])
                y = opool.tile([P, out_chunk], FP32)
                nc.scalar.activation(y[:], h[:],
                                     mybir.ActivationFunctionType.Identity,
                                     bias=b_ap, scale=s_ap)

                store.dma_start(out=out[:, i * out_chunk:(i + 1) * out_chunk],
                                in_=y[:])

    # run Bacc's legalization passes (multi-wait splitting, reg alloc, ...);
    # run_bass_via_pjrt serializes nc.m as-is and never finalizes.
    nc.finalize()
    return nc


def _get_nc():
    global _nc_cache
    if _nc_cache is None:
        _nc_cache = _build()
    return _nc_cache


def _make_in_maps(x, weight, bias):
    x = np.ascontiguousarray(np.asarray(x, dtype=np.float32))
    weight = np.asarray(weight, dtype=np.float32).reshape(C)
    bias = np.asarray(bias, dtype=np.float32).reshape(C)
    affine = np.stack([np.tile(weight * 0.25, B_LOC),
                       np.tile(bias, B_LOC)], axis=1)
    affine = np.ascontiguousarray(affine, dtype=np.float32)  # [P, 2]
    in_maps = []
    for k in range(N_CORES):
        shard = np.ascontiguousarray(
            x[k * B_LOC:(k + 1) * B_LOC].reshape(P, IMG))
        in_maps.append({"x": shard, "affine": affine})
    return in_maps


def run_sharded(x, weight, bias, trace=False, **kw):
    """Run the SPMD kernel; returns (full_output, BassKernelResults)."""
    res = run_bass_kernel_spmd(_get_nc(), _make_in_maps(x, weight, bias),
                               core_ids=list(range(N_CORES)), trace=trace, **kw)
    outs = [res.results[k]["out"].reshape(B_LOC, C, OS, OS)
            for k in range(N_CORES)]
    return np.concatenate(outs, axis=0), res


def kernel(x, weight, bias):
    out, _ = run_sharded(x, weight, bias, trace=False)
    return out
